# revision 29
# baseline (speedup 1.0000x reference)
"""GAT (2-layer dense-graph attention over 4096 nodes) as a Trainium2
Bass/Tile SPMD kernel across 8 NeuronCores.

Sharding: layer-0 attention destination rows are sharded 512/core. Each
core computes the full source-side quantities (h', d) from the full x and
s-scores for its own 512 destination rows. Layer 1 is sharded by SOURCE
rows instead: each core owns the 512 h1 rows it just produced (no h1
AllGather at all), computes partial softmax numerators/denominators for
ALL 4096 destinations over its source shard, and one ReduceScatter of the
[8*33, 512] partials delivers each core its own destination chunk summed.
The only other collective is an AllGather of the per-node s1 score row.

Math (exact softmax algebra): with z = s_i + d_j,
E = exp(leakyrelu(z)) = max(e^z, e^{0.2 z}). Softmax rows are invariant
to any per-i factor, so scale by e^{-0.2 s_i}:
E' = max(e^{0.8 s_i} e^{d_j}, e^{0.2 d_j}) = e^{d_j} * E'' with
E'' = max(e^{0.8 s_i}, e^{-0.8 d_j}).
The per-j factor e^{d_j} commutes into the matmul STATIONARY operand
(h'_j rows pre-scaled by e^{d_j}; denominator column holds e^{d_j}), so
the per-tile moving operand is ONE single-op elementwise max of the
broadcast e^{0.8 s} tile against the per-partition scalar e^{-0.8 d_j}.
Most e-tiles run on DVE (tensor_scalar_max, 2x_1P — the PTR scalar
occupies the second read port so 4x is impossible); a fraction runs on
GpSimd as tensor_tensor-max with a stride-0 broadcast AP to widen the
elementwise lane. BatchNorm (eval) is folded into weights host-side;
b0/b1 are zeros by construction of the problem and are dropped. x is
pre-transposed host-side and the [32,512] output block is transposed
host-side (pure data marshaling).

Scheduling notes:
- matmul start=True resets PSUM accumulation flags BANK-wide; a
  start=False write to a flag-cleared region overwrites. Hence p34
  (layer-1 projections incl the s1 column) accumulates in ONE bank with
  a single global start=True and region-wise stops.
- per-chunk softmax normalization is split: the denominator reciprocal
  issues at chunk end (DVE), everything that *waits* on it (the PE
  broadcast matmuls etc.) is deferred into the middle of the
  next-but-one head so the PE FIFO never stalls on the DVE.
- a keepalive matmul chain bridges the s1-AllGather window to keep the
  HAM clock gate at full rate.
"""

import numpy as np
import ml_dtypes

import concourse.bacc as bacc
import concourse.mybir as mybir
import concourse.tile as tile
from concourse import masks
from concourse.bass import broadcast_tensor_aps
from concourse.bass_utils import run_bass_kernel_spmd

F32 = mybir.dt.float32
BF16 = mybir.dt.bfloat16
ALU = mybir.AluOpType
ACT = mybir.ActivationFunctionType
N = 4096
NCORES = 8
RPC = N // NCORES          # destination rows per core = 512
NJT = N // 128             # 32 j-tiles of 128 source rows
NJT1 = RPC // 128          # 4 local j-tiles for layer 1
BN_EPS = 1e-5

_CACHE = {}


def _build():
    nc = bacc.Bacc("TRN2", target_bir_lowering=False, debug=False,
                   num_devices=NCORES)

    xt_d = nc.dram_tensor("xt33", [33, N], BF16, kind="ExternalInput")
    xst_d = nc.dram_tensor("xst33", [33, RPC], BF16, kind="ExternalInput")
    w0all_d = nc.dram_tensor("w0all", [33, 80], BF16, kind="ExternalInput")
    w0s_d = nc.dram_tensor("w0s", [33, 8], BF16, kind="ExternalInput")
    w1allh_d = nc.dram_tensor("w1allh", [8, 8 * 34], BF16, kind="ExternalInput")
    w1ones_d = nc.dram_tensor("w1ones", [1, 34], BF16, kind="ExternalInput")
    sela_d = nc.dram_tensor("sela", [8, 8 * 128], BF16, kind="ExternalInput")
    out_d = nc.dram_tensor("out", [32, RPC], F32, kind="ExternalOutput")

    with tile.TileContext(nc) as tc:
        with (
            tc.tile_pool(name="const", bufs=1) as const,
            tc.tile_pool(name="persist", bufs=1) as per,
            tc.tile_pool(name="dram", bufs=1, space="DRAM") as dram,
        ):
            # warmup fodder memsets come absolutely first so the PE
            # warm-up burst can start immediately
            wsrc = const.tile([128, 512], BF16)
            nc.vector.memset(wsrc[:], 0.5)
            wlhs = const.tile([128, 128], BF16)
            nc.vector.memset(wlhs[:], 0.25)
            ones_row = const.tile([1, 128], F32)
            nc.vector.memset(ones_row[:], 1.0)
            ones_row_bf = const.tile([1, 128], BF16)
            nc.vector.memset(ones_row_bf[:], 1.0)
            ident = const.tile([128, 128], F32)
            sela = const.tile([8, 8 * 128], BF16)
            nc.sync.dma_start(sela[:], sela_d[:])

            w0all = const.tile([33, 80], BF16)
            nc.sync.dma_start(w0all[:], w0all_d[:])
            w0s = const.tile([33, 8], BF16)
            nc.sync.dma_start(w0s[:], w0s_d[:])
            w1allh = const.tile([8, 8, 34], BF16)
            nc.sync.dma_start(
                w1allh[:], w1allh_d[:].rearrange("p (h c) -> p h c", h=8))
            w1ones = const.tile([1, 34], BF16)
            nc.sync.dma_start(w1ones[:], w1ones_d[:])

            # big persistent sbuf tensors
            xT = per.tile([33, N], BF16)       # x^T plus ones row
            xsT = per.tile([33, RPC], BF16)    # x_slice^T plus ones row
            # layer-0 stationary per (jt, h): scaled-hi 0:8, e^{d} at 32
            hpa0 = per.tile([128, NJT, 8, 33], BF16)
            d0r = per.tile([128, NJT, 8], F32)       # e^{-0.8 d0}
            nd0r = per.tile([128, NJT, 8], F32)      # -e^{-0.8 d0}
            atile = per.tile([128, 8, 512], BF16)    # e^{0.8 s0} bcast
            contc = per.tile([8, 8, 512], BF16)      # h1 local: [o, h, i]
            nrm = per.tile([8, 2, 512], F32)         # per-chunk normalized
            eneg = per.tile([8, 2, 512], F32)
            den2 = [per.tile([1, 2, 512], F32, name=f"den2_{c}")
                    for c in range(4)]
            lnden = [per.tile([1, 2, 512], F32, name=f"lnden_{c}")
                     for c in range(4)]
            rden2 = [per.tile([1, 2, 512], BF16, name=f"rden2_{c}")
                     for c in range(4)]
            # layer-1 stationary per jt: scaled-hi 0:32, e^{d1} at 32
            stat1 = per.tile([128, NJT1, 33], BF16)
            d1r = per.tile([128, NJT1], F32)         # e^{-0.8 d1}
            atile1 = per.tile([128, 8, 512], BF16)   # e^{0.8 s1} bcast
            s1loc = per.tile([128, NJT1], F32)
            s1row = per.tile([NJT1, 128], F32)
            s1g = per.tile([1, 8, 512], F32)
            a1rows = per.tile([1, 8, 512], BF16)
            rsb = per.tile([33, 512], F32)
            rden1 = per.tile([1, 512], BF16)
            rscr1 = per.tile([1, 512], F32)
            lnden1 = per.tile([1, 512], F32)
            norm1 = per.tile([32, 512], F32)

            s1d = dram.tile([NJT1, 128], F32, name="s1d", tag="s1d")
            s1gd = dram.tile([NCORES * NJT1, 128], F32, name="s1gd",
                             tag="s1gd")
            rsin = dram.tile([NCORES * 33, 512], F32, name="rsin", tag="rsin")
            rsout = dram.tile([33, 512], F32, name="rsout", tag="rsout")

            # ---------------- Phase A: projections -----------------
            with (
                tc.tile_pool(name="ld", bufs=2) as ld,
                tc.tile_pool(name="mm80", bufs=2, space="PSUM") as mm80,
                tc.tile_pool(name="pssa0", bufs=1, space="PSUM") as pssa0,
                tc.tile_pool(name="pssa", bufs=2, space="PSUM") as pssa,
            ):
                # PE warm-up burst: back-to-back matmuls flip the HAM
                # clock gate to 8/8 while input DMAs are still in flight
                wps = pssa0.tile([128, 512], F32, tag="wps")
                for r in range(20):
                    nc.tensor.matmul(wps[:], wlhs[:], wsrc[:],
                                     start=(r == 0), stop=(r == 19))

                nc.sync.dma_start(xT[:], xt_d[:])
                nc.sync.dma_start(xsT[:], xst_d[:])

                # s0 for this core's 512 dst rows; atile = e^{0.8 s0} bcast
                ps0 = pssa0.tile([8, 512], F32, tag="ps0")
                nc.tensor.matmul(ps0[:], w0s[:], xsT[:])
                a0row = ld.tile([8, 512], BF16, tag="a0row")
                nc.scalar.activation(a0row[:], ps0[:], ACT.Exp, scale=0.8)
                for h in range(8):
                    pa = pssa.tile([128, 512], F32, tag="pa")
                    nc.tensor.matmul(pa[:], sela[:, h * 128:(h + 1) * 128],
                                     a0row[:])
                    nc.scalar.copy(atile[:, h, :], pa[:])

                # h'0 scaled by e^{d0}, d0 exps, per 4-jt group
                for g in range(NJT // 4):
                    p80 = mm80.tile([128, 4, 80], F32)
                    for k in range(4):
                        jt = g * 4 + k
                        nc.tensor.matmul(p80[:, k, :],
                                         xT[:, jt * 128:(jt + 1) * 128],
                                         w0all[:])
                    # e^{d0} -> stationary col 32 (bf16), e^{-0.8 d0} -> d0r
                    nc.scalar.activation(
                        hpa0[:, g * 4:(g + 1) * 4, :, 32:33],
                        p80[:, :, 64:72], ACT.Exp)
                    nc.scalar.activation(
                        d0r[:, g * 4:(g + 1) * 4, :],
                        p80[:, :, 64:72], ACT.Exp, scale=-0.8)
                    nc.vector.tensor_scalar_mul(
                        nd0r[:, g * 4:(g + 1) * 4, :],
                        d0r[:, g * 4:(g + 1) * 4, :], -1.0)
                    for k in range(4):
                        jt = g * 4 + k
                        hsrc = p80[:, k, 0:64].rearrange("p (h o) -> p h o",
                                                         h=8)
                        sc_in, sc_b = broadcast_tensor_aps(
                            hsrc, hpa0[:, jt, :, 32:33])
                        nc.vector.tensor_tensor(hpa0[:, jt, :, 0:8],
                                                sc_in, sc_b, op=ALU.mult)

            # ------- Phase B: layer-0 attention + local normalize -------
            with (
                tc.tile_pool(name="epool", bufs=32) as epool,
                tc.tile_pool(name="nchunk", bufs=1) as nchunk,
                tc.tile_pool(name="agg", bufs=3, space="PSUM") as agg,
                tc.tile_pool(name="prb", bufs=1, space="PSUM") as prb,
                tc.tile_pool(name="p34p", bufs=1, space="PSUM") as p34p,
                tc.tile_pool(name="kap0", bufs=1, space="PSUM") as kap0,
            ):
                p34 = p34p.tile([128, NJT1, 34], F32, tag="p34")
                masks.make_identity(nc, ident[:])

                def keepalive(n, dep_ap):
                    ka = nchunk.tile([1, 128], BF16, tag="ka")
                    nc.scalar.copy(ka[:], dep_ap)
                    kps = kap0.tile([128, 512], F32, tag="kps")
                    for r in range(n):
                        nc.tensor.matmul(kps[:], ka[:], wsrc[0:1, :],
                                         start=(r == 0), stop=(r == n - 1))
                pgs = {}
                p34_started = [False]

                def norm_part_a(ch):
                    # only the reciprocal chain: runs on SE+DVE, nothing
                    # queued on the PE yet
                    pg_e, pg_o = pgs[2 * ch], pgs[2 * ch + 1]
                    nc.scalar.copy(den2[ch][:, 0, :], pg_e[32:33, :])
                    nc.scalar.copy(den2[ch][:, 1, :], pg_o[32:33, :])
                    nc.scalar.activation(lnden[ch][:], den2[ch][:], ACT.Ln)
                    nc.scalar.activation(rden2[ch][:], lnden[ch][:],
                                         ACT.Exp, scale=-1.0)

                def norm_part_b(ch):
                    # everything downstream of the reciprocal; issued one
                    # head later so the prb matmuls never block the PE
                    pg_e, pg_o = pgs[2 * ch], pgs[2 * ch + 1]
                    prb_e = prb.tile([8, 512], F32)
                    nc.tensor.matmul(prb_e[:], ones_row_bf[0:1, 0:8],
                                     rden2[ch][:, 0, :])
                    prb_o = prb.tile([8, 512], F32)
                    nc.tensor.matmul(prb_o[:], ones_row_bf[0:1, 0:8],
                                     rden2[ch][:, 1, :])
                    numc = nchunk.tile([8, 2, 512], F32, tag="numc")
                    nc.scalar.copy(numc[:, 0, :], pg_e[0:8, :])
                    nc.scalar.copy(numc[:, 1, :], pg_o[0:8, :])
                    prbs = nchunk.tile([8, 2, 512], F32, tag="prbs")
                    nc.scalar.copy(prbs[:, 0, :], prb_e[:])
                    nc.scalar.copy(prbs[:, 1, :], prb_o[:])
                    eng_tt = nc.vector if ch == 3 else nc.gpsimd
                    eng_tt.tensor_tensor(nrm[:], numc[:], prbs[:],
                                         op=ALU.mult)
                    # elu = (exp(-relu(-x)) - 1) + relu(x)
                    nc.scalar.activation(eneg[:], nrm[:], ACT.Relu,
                                         scale=-1.0)
                    nc.scalar.activation(eneg[:], eneg[:], ACT.Exp,
                                         scale=-1.0)
                    ppos = nchunk.tile([8, 2, 512], F32, tag="ppos")
                    nc.scalar.activation(ppos[:], nrm[:], ACT.Relu)
                    nc.vector.scalar_tensor_tensor(
                        contc[:, 2 * ch:2 * ch + 2, :], eneg[:], -1.0,
                        ppos[:], op0=ALU.add, op1=ALU.add)
                    # layer-1 projections (incl s1 col 33); single bank,
                    # single global start, region-wise stop
                    for jt in range(NJT1):
                        if ch == 0:
                            st = not p34_started[0]
                            p34_started[0] = True
                            nc.tensor.matmul(
                                p34[:, jt, :], ones_row_bf[:],
                                w1ones[:], start=st, stop=False)
                        for hc in (2 * ch, 2 * ch + 1):
                            nc.tensor.matmul(
                                p34[:, jt, :],
                                contc[:, hc, jt * 128:(jt + 1) * 128],
                                w1allh[:, hc, :],
                                start=False, stop=(ch == 3 and
                                                   hc == 2 * ch + 1))

                for h in range(8):
                    ch, hh = h // 2, h % 2
                    pg = agg.tile([33, 512], F32)
                    pgs[h] = pg
                    for jt in range(NJT):
                        if hh == 0 and h >= 2 and jt == 12:
                            norm_part_b(ch - 1)
                        e = epool.tile([128, 512], BF16, tag="e")
                        if jt % 6 == 3:
                            # max(a, r) = relu(a - r) + r on ScalarE
                            nc.scalar.activation(e[:], atile[:, h, :],
                                                 ACT.Relu,
                                                 bias=nd0r[:, jt, h:h + 1])
                            nc.scalar.activation(e[:], e[:], ACT.Identity,
                                                 bias=d0r[:, jt, h:h + 1])
                        else:
                            nc.vector.tensor_scalar_max(
                                e[:], atile[:, h, :], d0r[:, jt, h:h + 1])
                        nc.tensor.matmul(pg[:], hpa0[:, jt, h, 0:33], e[:],
                                         start=(jt == 0), stop=(jt == NJT - 1))
                    if hh == 1:
                        norm_part_a(ch)
                        if ch == 3:
                            keepalive(14, den2[3][:, 0, 0:128])
                norm_part_b(3)

                # s1 row extraction: col 33 of p34 -> [4,128] row-major,
                # then a contiguous AllGather
                for jt in range(NJT1):
                    nc.scalar.copy(s1loc[:, jt:jt + 1], p34[:, jt, 33:34])
                ts1 = prb.tile([NJT1, 128], F32, tag="ts1")
                nc.tensor.matmul(ts1[:], s1loc[:], ident[:],
                                 is_transpose=True)
                nc.scalar.copy(s1row[:], ts1[:])
                nc.sync.dma_start(s1d[:], s1row[:])
                keepalive(40, s1row[0:1, :])
                nc.gpsimd.collective_compute(
                    "AllGather",
                    ALU.bypass,
                    replica_groups=[list(range(NCORES))],
                    ins=[s1d.opt()],
                    outs=[s1gd.opt()],
                )
                nc.sync.dma_start(s1g[:], s1gd[:].rearrange("a b -> (a b)"))

                # layer-1 stationary: scaled-hi + e^{d1} col, d1r
                for jt in range(NJT1):
                    nc.scalar.activation(stat1[:, jt, 32:33],
                                         p34[:, jt, 32:33], ACT.Exp)
                    nc.scalar.activation(d1r[:, jt:jt + 1],
                                         p34[:, jt, 32:33],
                                         ACT.Exp, scale=-0.8)
                    sc_in, sc_b = broadcast_tensor_aps(
                        p34[:, jt, 0:32], stat1[:, jt, 32:33])
                    nc.vector.tensor_tensor(stat1[:, jt, 0:32],
                                            sc_in, sc_b, op=ALU.mult)

            # ---------------- Phase D: layer 1 ----------------
            with (
                tc.tile_pool(name="e1pool", bufs=16) as e1pool,
                tc.tile_pool(name="pa1p", bufs=2, space="PSUM") as pa1p,
                tc.tile_pool(name="agg1", bufs=2, space="PSUM") as agg1,
                tc.tile_pool(name="prb1p", bufs=1, space="PSUM") as prb1p,
                tc.tile_pool(name="rslp", bufs=2) as rslp,
            ):
                nc.scalar.activation(a1rows[:], s1g[:], ACT.Exp, scale=0.8)
                for c in range(8):
                    pa1 = pa1p.tile([128, 512], F32)
                    nc.tensor.matmul(pa1[:], ones_row_bf[:],
                                     a1rows[:, c, :])
                    nc.scalar.copy(atile1[:, c, :], pa1[:])

                for c in range(8):
                    pg1 = agg1.tile([33, 512], F32)
                    for jt in range(NJT1):
                        e1 = e1pool.tile([128, 512], BF16, tag="e1")
                        nc.vector.tensor_scalar_max(
                            e1[:], atile1[:, c, :], d1r[:, jt:jt + 1])
                        nc.tensor.matmul(pg1[:], stat1[:, jt, 0:33], e1[:],
                                         start=(jt == 0),
                                         stop=(jt == NJT1 - 1))
                    rsl = rslp.tile([33, 512], F32, tag="rsl")
                    nc.scalar.copy(rsl[:], pg1[:])
                    nc.sync.dma_start(rsin[c * 33:(c + 1) * 33, :], rsl[:])

                nc.gpsimd.collective_compute(
                    "ReduceScatter",
                    ALU.add,
                    replica_groups=[list(range(NCORES))],
                    ins=[rsin.opt()],
                    outs=[rsout.opt()],
                )
                nc.sync.dma_start(rsb[:], rsout[:])
                nc.scalar.copy(rscr1[:], rsb[32:33, :])
                nc.scalar.activation(lnden1[:], rscr1[:], ACT.Ln)
                nc.scalar.activation(rden1[:], lnden1[:], ACT.Exp,
                                     scale=-1.0)
                prb1 = prb1p.tile([32, 512], F32, tag="prb1")
                nc.tensor.matmul(prb1[:], ones_row_bf[0:1, 0:32], rden1[:])
                nc.vector.tensor_tensor(norm1[:], rsb[0:32, :], prb1[:],
                                        op=ALU.mult)
                nc.sync.dma_start(out_d[:], norm1[:])

    nc.compile()
    return nc


def _fold(inputs):
    """Host-side BN folding and attention-projection folding (numpy)."""
    f64 = np.float64
    x = np.asarray(inputs["x"], np.float32)
    w0 = np.asarray(inputs["w0"], f64)          # [8, 32, 8]
    w1 = np.asarray(inputs["w1"], f64)          # [1, 64, 32]
    a_src0 = np.asarray(inputs["a_src0"], f64)[..., 0]   # [8, 8]
    a_dst0 = np.asarray(inputs["a_dst0"], f64)[..., 0]   # [8, 8]
    a_src1 = np.asarray(inputs["a_src1"], f64)[0, :, 0]  # [32]
    a_dst1 = np.asarray(inputs["a_dst1"], f64)[0, :, 0]  # [32]

    al0 = np.asarray(inputs["bn0_gamma"], f64) / np.sqrt(
        np.asarray(inputs["bn0_var"], f64) + BN_EPS)
    sh0 = np.asarray(inputs["bn0_beta"], f64) - \
        np.asarray(inputs["bn0_mean"], f64) * al0
    al1 = np.asarray(inputs["bn1_gamma"], f64) / np.sqrt(
        np.asarray(inputs["bn1_var"], f64) + BN_EPS)
    sh1 = np.asarray(inputs["bn1_beta"], f64) - \
        np.asarray(inputs["bn1_mean"], f64) * al1

    # layer 0 folds
    w0flat = (al0[None, :, None] * w0).transpose(1, 0, 2).reshape(32, 64)
    beta0h = np.einsum("i,hio->ho", sh0, w0)     # [8, 8]
    beta0 = beta0h.reshape(64)
    as0 = al0[:, None] * np.einsum("hio,ho->ih", w0, a_src0)   # [32, 8]
    sb0 = np.einsum("ho,ho->h", beta0h, a_src0)
    ad0 = al0[:, None] * np.einsum("hio,ho->ih", w0, a_dst0)
    db0 = np.einsum("ho,ho->h", beta0h, a_dst0)

    w0all = np.zeros((33, 80), f64)
    w0all[0:32, 0:64] = w0flat
    w0all[32, 0:64] = beta0
    w0all[0:32, 64:72] = ad0
    w0all[32, 64:72] = db0
    w0s = np.zeros((33, 8), f64)
    w0s[0:32, :] = as0
    w0s[32, :] = sb0

    # layer 1 folds (feature order f = h*8 + o to match contc [o, h, i])
    w1m = w1[0]                                   # [64, 32]
    w1flat = al1[:, None] * w1m
    beta1 = sh1 @ w1m                             # [32]
    as1 = al1 * (w1m @ a_src1)
    sb1 = beta1 @ a_src1
    ad1 = al1 * (w1m @ a_dst1)
    db1 = beta1 @ a_dst1

    w1allh = np.zeros((8, 8, 34), f64)            # [o, h, col]
    for h in range(8):
        w1allh[:, h, 0:32] = w1flat[h * 8:(h + 1) * 8]
        w1allh[:, h, 32] = ad1[h * 8:(h + 1) * 8]
        w1allh[:, h, 33] = as1[h * 8:(h + 1) * 8]
    w1ones = np.zeros((1, 34), f64)
    w1ones[0, 0:32] = beta1
    w1ones[0, 32] = db1
    w1ones[0, 33] = sb1

    sela = np.zeros((8, 8, 128), ml_dtypes.bfloat16)  # row h ones in block h
    for h in range(8):
        sela[h, h, :] = 1.0

    xt33 = np.concatenate(
        [x, np.ones((x.shape[0], 1), np.float32)], 1).T

    bf = ml_dtypes.bfloat16
    return {
        "xt33": np.ascontiguousarray(xt33.astype(bf)),
        "w0all": w0all.astype(bf),
        "w0s": w0s.astype(bf),
        "w1allh": w1allh.reshape(8, 8 * 34).astype(bf),
        "w1ones": w1ones.astype(bf),
        "sela": sela.reshape(8, 8 * 128),
    }


def kernel(**inputs) -> np.ndarray:
    if "nc" not in _CACHE:
        _CACHE["nc"] = _build()
    nc = _CACHE["nc"]

    shared = _fold(inputs)
    xt33 = shared["xt33"]
    in_maps = []
    for c in range(NCORES):
        m = dict(shared)
        m["xst33"] = np.ascontiguousarray(xt33[:, c * RPC:(c + 1) * RPC])
        in_maps.append(m)

    res = run_bass_kernel_spmd(nc, in_maps, list(range(NCORES)))
    out = np.concatenate(
        [np.ascontiguousarray(res.results[c]["out"].T)
         for c in range(NCORES)], axis=0)
    return out.astype(np.float32)


# revision 31
# speedup vs baseline: 1.1763x; 1.1763x over previous
"""GAT (2-layer dense-graph attention over 4096 nodes) as a Trainium2
Bass/Tile SPMD kernel across 8 NeuronCores.

Sharding: layer-0 attention destination rows are sharded 512/core. Each
core computes the full source-side quantities (h', d) from the full x and
s-scores for its own 512 destination rows. Layer 1 is sharded by SOURCE
rows instead: each core owns the 512 h1 rows it just produced (no h1
AllGather at all), computes partial softmax numerators/denominators for
ALL 4096 destinations over its source shard, and one ReduceScatter of the
[8*33, 512] partials delivers each core its own destination chunk summed.
The only other collective is an AllGather of the per-node s1 score row.

Math (exact softmax algebra): with z = s_i + d_j,
E = exp(leakyrelu(z)) = max(e^z, e^{0.2 z}). Softmax rows are invariant
to any per-i factor, so scale by e^{-0.2 s_i}:
E' = max(e^{0.8 s_i} e^{d_j}, e^{0.2 d_j}) = e^{d_j} * E'' with
E'' = max(e^{0.8 s_i}, e^{-0.8 d_j}).
The per-j factor e^{d_j} commutes into the matmul STATIONARY operand
(h'_j rows pre-scaled by e^{d_j}; denominator column holds e^{d_j}), so
the per-tile moving operand is ONE single-op elementwise max of the
broadcast e^{0.8 s} tile against the per-partition scalar e^{-0.8 d_j}.
Most e-tiles run on DVE (tensor_scalar_max, 2x_1P — the PTR scalar
occupies the second read port so 4x is impossible); a fraction runs on
GpSimd as tensor_tensor-max with a stride-0 broadcast AP to widen the
elementwise lane. BatchNorm (eval) is folded into weights host-side;
b0/b1 are zeros by construction of the problem and are dropped. x is
pre-transposed host-side and the [32,512] output block is transposed
host-side (pure data marshaling).

Scheduling notes:
- matmul start=True resets PSUM accumulation flags BANK-wide; a
  start=False write to a flag-cleared region overwrites. Hence p34
  (layer-1 projections incl the s1 column) accumulates in ONE bank with
  a single global start=True and region-wise stops.
- per-chunk softmax normalization is split: the denominator reciprocal
  issues at chunk end (DVE), everything that *waits* on it (the PE
  broadcast matmuls etc.) is deferred into the middle of the
  next-but-one head so the PE FIFO never stalls on the DVE.
- a keepalive matmul chain bridges the s1-AllGather window to keep the
  HAM clock gate at full rate.
"""

import numpy as np
import ml_dtypes

import concourse.bacc as bacc
import concourse.mybir as mybir
import concourse.tile as tile
from concourse import masks
from concourse.bass import broadcast_tensor_aps
from concourse.bass_utils import run_bass_kernel_spmd

F32 = mybir.dt.float32
BF16 = mybir.dt.bfloat16
ALU = mybir.AluOpType
ACT = mybir.ActivationFunctionType
N = 4096
NCORES = 8
RPC = N // NCORES          # destination rows per core = 512
NJT = N // 128             # 32 j-tiles of 128 source rows
NJT1 = RPC // 128          # 4 local j-tiles for layer 1
BN_EPS = 1e-5

_CACHE = {}


def _build():
    nc = bacc.Bacc("TRN2", target_bir_lowering=False, debug=False,
                   num_devices=NCORES)

    xt_d = nc.dram_tensor("xt33", [33, N], BF16, kind="ExternalInput")
    xst_d = nc.dram_tensor("xst33", [33, RPC], BF16, kind="ExternalInput")
    w0all_d = nc.dram_tensor("w0all", [33, 80], BF16, kind="ExternalInput")
    w0s_d = nc.dram_tensor("w0s", [33, 8], BF16, kind="ExternalInput")
    w1allh_d = nc.dram_tensor("w1allh", [8, 8 * 34], BF16, kind="ExternalInput")
    w1ones_d = nc.dram_tensor("w1ones", [1, 34], BF16, kind="ExternalInput")
    sela_d = nc.dram_tensor("sela", [8, 8 * 128], BF16, kind="ExternalInput")
    out_d = nc.dram_tensor("out", [32, RPC], F32, kind="ExternalOutput")

    with tile.TileContext(nc) as tc:
        with (
            tc.tile_pool(name="const", bufs=1) as const,
            tc.tile_pool(name="persist", bufs=1) as per,
            tc.tile_pool(name="dram", bufs=1, space="DRAM") as dram,
        ):
            # warmup fodder memsets come absolutely first so the PE
            # warm-up burst can start immediately
            wsrc = const.tile([128, 512], BF16)
            nc.vector.memset(wsrc[:], 0.5)
            wlhs = const.tile([128, 128], BF16)
            nc.vector.memset(wlhs[:], 0.25)
            ones_row = const.tile([1, 128], F32)
            nc.vector.memset(ones_row[:], 1.0)
            ones_row_bf = const.tile([1, 128], BF16)
            nc.vector.memset(ones_row_bf[:], 1.0)
            neg1c = const.tile([8, 1], F32)
            nc.vector.memset(neg1c[:], -1.0)
            ident = const.tile([128, 128], F32)
            sela = const.tile([8, 8 * 128], BF16)
            nc.sync.dma_start(sela[:], sela_d[:])

            w0all = const.tile([33, 80], BF16)
            nc.sync.dma_start(w0all[:], w0all_d[:])
            w0s = const.tile([33, 8], BF16)
            nc.sync.dma_start(w0s[:], w0s_d[:])
            w1allh = const.tile([8, 8, 34], BF16)
            nc.sync.dma_start(
                w1allh[:], w1allh_d[:].rearrange("p (h c) -> p h c", h=8))
            w1ones = const.tile([1, 34], BF16)
            nc.sync.dma_start(w1ones[:], w1ones_d[:])

            # big persistent sbuf tensors
            xT = per.tile([33, N], BF16)       # x^T plus ones row
            xsT = per.tile([33, RPC], BF16)    # x_slice^T plus ones row
            # layer-0 stationary per (jt, h): scaled-hi 0:8, e^{d} at 32
            hpa0 = per.tile([128, NJT, 8, 33], BF16)
            d0r = per.tile([128, NJT, 8], F32)       # e^{-0.8 d0}
            atile = per.tile([128, 8, 512], BF16)    # e^{0.8 s0} bcast
            contc = per.tile([8, 8, 512], BF16)      # h1 local: [o, h, i]
            nrm = per.tile([8, 2, 512], F32)         # per-chunk normalized
            eneg = per.tile([8, 2, 512], F32)
            den2 = [per.tile([1, 2, 512], F32, name=f"den2_{c}")
                    for c in range(4)]
            lnden = [per.tile([1, 2, 512], F32, name=f"lnden_{c}")
                     for c in range(4)]
            rden2 = [per.tile([1, 2, 512], BF16, name=f"rden2_{c}")
                     for c in range(4)]
            # layer-1 stationary per jt: scaled-hi 0:32, e^{d1} at 32
            stat1 = per.tile([128, NJT1, 33], BF16)
            d1r = per.tile([128, NJT1], F32)         # e^{-0.8 d1}
            atile1 = per.tile([128, 8, 512], BF16)   # e^{0.8 s1} bcast
            s1loc = per.tile([128, NJT1], F32)
            s1row = per.tile([NJT1, 128], F32)
            s1g = per.tile([1, 8, 512], F32)
            a1rows = per.tile([1, 8, 512], BF16)
            rsb = per.tile([33, 512], F32)
            rden1 = per.tile([1, 512], BF16)
            rscr1 = per.tile([1, 512], F32)
            lnden1 = per.tile([1, 512], F32)
            norm1 = per.tile([32, 512], F32)

            s1d = dram.tile([NJT1, 128], F32, name="s1d", tag="s1d")
            s1gd = dram.tile([NCORES * NJT1, 128], F32, name="s1gd",
                             tag="s1gd")
            rsin = dram.tile([NCORES * 33, 512], F32, name="rsin", tag="rsin")
            rsout = dram.tile([33, 512], F32, name="rsout", tag="rsout")

            # ---------------- Phase A: projections -----------------
            with (
                tc.tile_pool(name="ld", bufs=2) as ld,
                tc.tile_pool(name="mm80", bufs=2, space="PSUM") as mm80,
                tc.tile_pool(name="pssa0", bufs=1, space="PSUM") as pssa0,
                tc.tile_pool(name="pssa", bufs=2, space="PSUM") as pssa,
            ):
                # PE warm-up burst: back-to-back matmuls flip the HAM
                # clock gate to 8/8 while input DMAs are still in flight
                wps = pssa0.tile([128, 512], F32, tag="wps")
                for r in range(20):
                    nc.tensor.matmul(wps[:], wlhs[:], wsrc[:],
                                     start=(r == 0), stop=(r == 19))

                nc.sync.dma_start(xT[:], xt_d[:])
                nc.sync.dma_start(xsT[:], xst_d[:])

                # s0 for this core's 512 dst rows; atile = e^{0.8 s0} bcast
                ps0 = pssa0.tile([8, 512], F32, tag="ps0")
                nc.tensor.matmul(ps0[:], w0s[:], xsT[:])
                a0row = ld.tile([8, 512], BF16, tag="a0row")
                nc.scalar.activation(a0row[:], ps0[:], ACT.Exp, scale=0.8)
                for h in range(8):
                    pa = pssa.tile([128, 512], F32, tag="pa")
                    nc.tensor.matmul(pa[:], sela[:, h * 128:(h + 1) * 128],
                                     a0row[:])
                    nc.scalar.copy(atile[:, h, :], pa[:])

                # h'0 scaled by e^{d0}, d0 exps, per 4-jt group
                for g in range(NJT // 4):
                    p80 = mm80.tile([128, 4, 80], F32)
                    for k in range(4):
                        jt = g * 4 + k
                        nc.tensor.matmul(p80[:, k, :],
                                         xT[:, jt * 128:(jt + 1) * 128],
                                         w0all[:])
                    # e^{d0} -> stationary col 32 (bf16), e^{-0.8 d0} -> d0r
                    nc.scalar.activation(
                        hpa0[:, g * 4:(g + 1) * 4, :, 32:33],
                        p80[:, :, 64:72], ACT.Exp)
                    nc.scalar.activation(
                        d0r[:, g * 4:(g + 1) * 4, :],
                        p80[:, :, 64:72], ACT.Exp, scale=-0.8)
                    for k in range(4):
                        jt = g * 4 + k
                        hsrc = p80[:, k, 0:64].rearrange("p (h o) -> p h o",
                                                         h=8)
                        sc_in, sc_b = broadcast_tensor_aps(
                            hsrc, hpa0[:, jt, :, 32:33])
                        nc.vector.tensor_tensor(hpa0[:, jt, :, 0:8],
                                                sc_in, sc_b, op=ALU.mult)

            # ------- Phase B: layer-0 attention + local normalize -------
            with (
                tc.tile_pool(name="epool", bufs=32) as epool,
                tc.tile_pool(name="nchunk", bufs=1) as nchunk,
                tc.tile_pool(name="agg", bufs=3, space="PSUM") as agg,
                tc.tile_pool(name="prb", bufs=1, space="PSUM") as prb,
                tc.tile_pool(name="p34p", bufs=1, space="PSUM") as p34p,
                tc.tile_pool(name="kap0", bufs=1, space="PSUM") as kap0,
            ):
                p34 = p34p.tile([128, NJT1, 34], F32, tag="p34")
                masks.make_identity(nc, ident[:])

                def keepalive(n, dep_ap):
                    ka = nchunk.tile([1, 128], BF16, tag="ka")
                    nc.scalar.copy(ka[:], dep_ap)
                    kps = kap0.tile([128, 512], F32, tag="kps")
                    for r in range(n):
                        nc.tensor.matmul(kps[:], ka[:], wsrc[0:1, :],
                                         start=(r == 0), stop=(r == n - 1))
                pgs = {}
                p34_started = [False]

                def norm_part_a(ch):
                    # only the reciprocal chain: runs on SE+DVE, nothing
                    # queued on the PE yet
                    pg_e, pg_o = pgs[2 * ch], pgs[2 * ch + 1]
                    nc.scalar.copy(den2[ch][:, 0, :], pg_e[32:33, :])
                    nc.scalar.copy(den2[ch][:, 1, :], pg_o[32:33, :])
                    nc.scalar.activation(lnden[ch][:], den2[ch][:], ACT.Ln)
                    nc.scalar.activation(rden2[ch][:], lnden[ch][:],
                                         ACT.Exp, scale=-1.0)

                def norm_part_b(ch):
                    # everything downstream of the reciprocal; issued one
                    # head later so the prb matmuls never block the PE
                    pg_e, pg_o = pgs[2 * ch], pgs[2 * ch + 1]
                    prb_e = prb.tile([8, 512], F32)
                    nc.tensor.matmul(prb_e[:], ones_row_bf[0:1, 0:8],
                                     rden2[ch][:, 0, :])
                    prb_o = prb.tile([8, 512], F32)
                    nc.tensor.matmul(prb_o[:], ones_row_bf[0:1, 0:8],
                                     rden2[ch][:, 1, :])
                    numc = nchunk.tile([8, 2, 512], F32, tag="numc")
                    nc.scalar.copy(numc[:, 0, :], pg_e[0:8, :])
                    nc.scalar.copy(numc[:, 1, :], pg_o[0:8, :])
                    nc.vector.tensor_tensor(nrm[:, 0, :], numc[:, 0, :],
                                            prb_e[:], op=ALU.mult)
                    nc.vector.tensor_tensor(nrm[:, 1, :], numc[:, 1, :],
                                            prb_o[:], op=ALU.mult)
                    # elu = (exp(-relu(-x)) - 1) + relu(x)
                    nc.scalar.activation(eneg[:], nrm[:], ACT.Relu,
                                         scale=-1.0)
                    nc.scalar.activation(eneg[:], eneg[:], ACT.Exp,
                                         scale=-1.0)
                    ppos = nchunk.tile([8, 2, 512], F32, tag="ppos")
                    nc.scalar.activation(ppos[:], nrm[:], ACT.Relu)
                    esum = nchunk.tile([8, 2, 512], F32, tag="esum")
                    nc.gpsimd.tensor_tensor(esum[:], eneg[:], ppos[:],
                                            op=ALU.add)
                    nc.scalar.activation(contc[:, 2 * ch:2 * ch + 2, :],
                                         esum[:], ACT.Identity,
                                         bias=neg1c[:])
                    # layer-1 projections (incl s1 col 33); single bank,
                    # single global start, region-wise stop
                    for jt in range(NJT1):
                        if ch == 0:
                            st = not p34_started[0]
                            p34_started[0] = True
                            nc.tensor.matmul(
                                p34[:, jt, :], ones_row_bf[:],
                                w1ones[:], start=st, stop=False)
                        for hc in (2 * ch, 2 * ch + 1):
                            nc.tensor.matmul(
                                p34[:, jt, :],
                                contc[:, hc, jt * 128:(jt + 1) * 128],
                                w1allh[:, hc, :],
                                start=False, stop=(ch == 3 and
                                                   hc == 2 * ch + 1))

                for h in range(8):
                    ch, hh = h // 2, h % 2
                    pg = agg.tile([33, 512], F32)
                    pgs[h] = pg
                    for jt in range(NJT):
                        if hh == 0 and h >= 2 and jt == 12:
                            norm_part_b(ch - 1)
                        e = epool.tile([128, 512], BF16, tag="e")
                        nc.vector.tensor_scalar_max(
                            e[:], atile[:, h, :], d0r[:, jt, h:h + 1])
                        nc.tensor.matmul(pg[:], hpa0[:, jt, h, 0:33], e[:],
                                         start=(jt == 0), stop=(jt == NJT - 1))
                    if hh == 1:
                        norm_part_a(ch)
                        if ch == 3:
                            keepalive(14, den2[3][:, 0, 0:128])
                norm_part_b(3)

                # s1 row extraction: col 33 of p34 -> [4,128] row-major,
                # then a contiguous AllGather
                for jt in range(NJT1):
                    nc.scalar.copy(s1loc[:, jt:jt + 1], p34[:, jt, 33:34])
                ts1 = prb.tile([NJT1, 128], F32, tag="ts1")
                nc.tensor.matmul(ts1[:], s1loc[:], ident[:],
                                 is_transpose=True)
                nc.scalar.copy(s1row[:], ts1[:])
                nc.sync.dma_start(s1d[:], s1row[:])
                keepalive(40, s1row[0:1, :])
                nc.gpsimd.collective_compute(
                    "AllGather",
                    ALU.bypass,
                    replica_groups=[list(range(NCORES))],
                    ins=[s1d.opt()],
                    outs=[s1gd.opt()],
                )
                nc.sync.dma_start(s1g[:], s1gd[:].rearrange("a b -> (a b)"))

                # layer-1 stationary: scaled-hi + e^{d1} col, d1r
                for jt in range(NJT1):
                    nc.scalar.activation(stat1[:, jt, 32:33],
                                         p34[:, jt, 32:33], ACT.Exp)
                    nc.scalar.activation(d1r[:, jt:jt + 1],
                                         p34[:, jt, 32:33],
                                         ACT.Exp, scale=-0.8)
                    sc_in, sc_b = broadcast_tensor_aps(
                        p34[:, jt, 0:32], stat1[:, jt, 32:33])
                    nc.vector.tensor_tensor(stat1[:, jt, 0:32],
                                            sc_in, sc_b, op=ALU.mult)

            # ---------------- Phase D: layer 1 ----------------
            with (
                tc.tile_pool(name="e1pool", bufs=16) as e1pool,
                tc.tile_pool(name="pa1p", bufs=2, space="PSUM") as pa1p,
                tc.tile_pool(name="agg1", bufs=2, space="PSUM") as agg1,
                tc.tile_pool(name="prb1p", bufs=1, space="PSUM") as prb1p,
                tc.tile_pool(name="rslp", bufs=2) as rslp,
            ):
                nc.scalar.activation(a1rows[:], s1g[:], ACT.Exp, scale=0.8)
                for c in range(8):
                    pa1 = pa1p.tile([128, 512], F32)
                    nc.tensor.matmul(pa1[:], ones_row_bf[:],
                                     a1rows[:, c, :])
                    nc.scalar.copy(atile1[:, c, :], pa1[:])

                for c in range(8):
                    pg1 = agg1.tile([33, 512], F32)
                    for jt in range(NJT1):
                        e1 = e1pool.tile([128, 512], BF16, tag="e1")
                        nc.vector.tensor_scalar_max(
                            e1[:], atile1[:, c, :], d1r[:, jt:jt + 1])
                        nc.tensor.matmul(pg1[:], stat1[:, jt, 0:33], e1[:],
                                         start=(jt == 0),
                                         stop=(jt == NJT1 - 1))
                    rsl = rslp.tile([33, 512], F32, tag="rsl")
                    nc.scalar.copy(rsl[:], pg1[:])
                    nc.sync.dma_start(rsin[c * 33:(c + 1) * 33, :], rsl[:])

                nc.gpsimd.collective_compute(
                    "ReduceScatter",
                    ALU.add,
                    replica_groups=[list(range(NCORES))],
                    ins=[rsin.opt()],
                    outs=[rsout.opt()],
                )
                nc.sync.dma_start(rsb[:], rsout[:])
                nc.scalar.copy(rscr1[:], rsb[32:33, :])
                nc.scalar.activation(lnden1[:], rscr1[:], ACT.Ln)
                nc.scalar.activation(rden1[:], lnden1[:], ACT.Exp,
                                     scale=-1.0)
                prb1 = prb1p.tile([32, 512], F32, tag="prb1")
                nc.tensor.matmul(prb1[:], ones_row_bf[0:1, 0:32], rden1[:])
                nc.vector.tensor_tensor(norm1[:], rsb[0:32, :], prb1[:],
                                        op=ALU.mult)
                nc.sync.dma_start(out_d[:], norm1[:])

    nc.compile()
    return nc


def _fold(inputs):
    """Host-side BN folding and attention-projection folding (numpy)."""
    f64 = np.float64
    x = np.asarray(inputs["x"], np.float32)
    w0 = np.asarray(inputs["w0"], f64)          # [8, 32, 8]
    w1 = np.asarray(inputs["w1"], f64)          # [1, 64, 32]
    a_src0 = np.asarray(inputs["a_src0"], f64)[..., 0]   # [8, 8]
    a_dst0 = np.asarray(inputs["a_dst0"], f64)[..., 0]   # [8, 8]
    a_src1 = np.asarray(inputs["a_src1"], f64)[0, :, 0]  # [32]
    a_dst1 = np.asarray(inputs["a_dst1"], f64)[0, :, 0]  # [32]

    al0 = np.asarray(inputs["bn0_gamma"], f64) / np.sqrt(
        np.asarray(inputs["bn0_var"], f64) + BN_EPS)
    sh0 = np.asarray(inputs["bn0_beta"], f64) - \
        np.asarray(inputs["bn0_mean"], f64) * al0
    al1 = np.asarray(inputs["bn1_gamma"], f64) / np.sqrt(
        np.asarray(inputs["bn1_var"], f64) + BN_EPS)
    sh1 = np.asarray(inputs["bn1_beta"], f64) - \
        np.asarray(inputs["bn1_mean"], f64) * al1

    # layer 0 folds
    w0flat = (al0[None, :, None] * w0).transpose(1, 0, 2).reshape(32, 64)
    beta0h = np.einsum("i,hio->ho", sh0, w0)     # [8, 8]
    beta0 = beta0h.reshape(64)
    as0 = al0[:, None] * np.einsum("hio,ho->ih", w0, a_src0)   # [32, 8]
    sb0 = np.einsum("ho,ho->h", beta0h, a_src0)
    ad0 = al0[:, None] * np.einsum("hio,ho->ih", w0, a_dst0)
    db0 = np.einsum("ho,ho->h", beta0h, a_dst0)

    w0all = np.zeros((33, 80), f64)
    w0all[0:32, 0:64] = w0flat
    w0all[32, 0:64] = beta0
    w0all[0:32, 64:72] = ad0
    w0all[32, 64:72] = db0
    w0s = np.zeros((33, 8), f64)
    w0s[0:32, :] = as0
    w0s[32, :] = sb0

    # layer 1 folds (feature order f = h*8 + o to match contc [o, h, i])
    w1m = w1[0]                                   # [64, 32]
    w1flat = al1[:, None] * w1m
    beta1 = sh1 @ w1m                             # [32]
    as1 = al1 * (w1m @ a_src1)
    sb1 = beta1 @ a_src1
    ad1 = al1 * (w1m @ a_dst1)
    db1 = beta1 @ a_dst1

    w1allh = np.zeros((8, 8, 34), f64)            # [o, h, col]
    for h in range(8):
        w1allh[:, h, 0:32] = w1flat[h * 8:(h + 1) * 8]
        w1allh[:, h, 32] = ad1[h * 8:(h + 1) * 8]
        w1allh[:, h, 33] = as1[h * 8:(h + 1) * 8]
    w1ones = np.zeros((1, 34), f64)
    w1ones[0, 0:32] = beta1
    w1ones[0, 32] = db1
    w1ones[0, 33] = sb1

    sela = np.zeros((8, 8, 128), ml_dtypes.bfloat16)  # row h ones in block h
    for h in range(8):
        sela[h, h, :] = 1.0

    xt33 = np.concatenate(
        [x, np.ones((x.shape[0], 1), np.float32)], 1).T

    bf = ml_dtypes.bfloat16
    return {
        "xt33": np.ascontiguousarray(xt33.astype(bf)),
        "w0all": w0all.astype(bf),
        "w0s": w0s.astype(bf),
        "w1allh": w1allh.reshape(8, 8 * 34).astype(bf),
        "w1ones": w1ones.astype(bf),
        "sela": sela.reshape(8, 8 * 128),
    }


def kernel(**inputs) -> np.ndarray:
    if "nc" not in _CACHE:
        _CACHE["nc"] = _build()
    nc = _CACHE["nc"]

    shared = _fold(inputs)
    xt33 = shared["xt33"]
    in_maps = []
    for c in range(NCORES):
        m = dict(shared)
        m["xst33"] = np.ascontiguousarray(xt33[:, c * RPC:(c + 1) * RPC])
        in_maps.append(m)

    res = run_bass_kernel_spmd(nc, in_maps, list(range(NCORES)))
    out = np.concatenate(
        [np.ascontiguousarray(res.results[c]["out"].T)
         for c in range(NCORES)], axis=0)
    return out.astype(np.float32)


# revision 32
# speedup vs baseline: 1.2426x; 1.0563x over previous
"""GAT (2-layer dense-graph attention over 4096 nodes) as a Trainium2
Bass/Tile SPMD kernel across 8 NeuronCores.

Sharding: layer-0 attention destination rows are sharded 512/core. Each
core computes the full source-side quantities (h', d) from the full x and
s-scores for its own 512 destination rows. Layer 1 is sharded by SOURCE
rows instead: each core owns the 512 h1 rows it just produced (no h1
AllGather at all), computes partial softmax numerators/denominators for
ALL 4096 destinations over its source shard, and one ReduceScatter of the
[8*33, 512] partials delivers each core its own destination chunk summed.
The only other collective is an AllGather of the per-node s1 score row.

Math (exact softmax algebra): with z = s_i + d_j,
E = exp(leakyrelu(z)) = max(e^z, e^{0.2 z}). Softmax rows are invariant
to any per-i factor, so scale by e^{-0.2 s_i}:
E' = max(e^{0.8 s_i} e^{d_j}, e^{0.2 d_j}) = e^{d_j} * E'' with
E'' = max(e^{0.8 s_i}, e^{-0.8 d_j}).
The per-j factor e^{d_j} commutes into the matmul STATIONARY operand
(h'_j rows pre-scaled by e^{d_j}; denominator column holds e^{d_j}), so
the per-tile moving operand is ONE single-op elementwise max of the
broadcast e^{0.8 s} tile against the per-partition scalar e^{-0.8 d_j}.
Most e-tiles run on DVE (tensor_scalar_max, 2x_1P — the PTR scalar
occupies the second read port so 4x is impossible); a fraction runs on
GpSimd as tensor_tensor-max with a stride-0 broadcast AP to widen the
elementwise lane. BatchNorm (eval) is folded into weights host-side;
b0/b1 are zeros by construction of the problem and are dropped. x is
pre-transposed host-side and the [32,512] output block is transposed
host-side (pure data marshaling).

Scheduling notes:
- matmul start=True resets PSUM accumulation flags BANK-wide; a
  start=False write to a flag-cleared region overwrites. Hence p34
  (layer-1 projections incl the s1 column) accumulates in ONE bank with
  a single global start=True and region-wise stops.
- per-chunk softmax normalization is split: the denominator reciprocal
  issues at chunk end (DVE), everything that *waits* on it (the PE
  broadcast matmuls etc.) is deferred into the middle of the
  next-but-one head so the PE FIFO never stalls on the DVE.
- a keepalive matmul chain bridges the s1-AllGather window to keep the
  HAM clock gate at full rate.
"""

import numpy as np
import ml_dtypes

import concourse.bacc as bacc
import concourse.mybir as mybir
import concourse.tile as tile
from concourse import masks
from concourse.bass import broadcast_tensor_aps
from concourse.bass_utils import run_bass_kernel_spmd

F32 = mybir.dt.float32
BF16 = mybir.dt.bfloat16
ALU = mybir.AluOpType
ACT = mybir.ActivationFunctionType
N = 4096
NCORES = 8
RPC = N // NCORES          # destination rows per core = 512
NJT = N // 128             # 32 j-tiles of 128 source rows
NJT1 = RPC // 128          # 4 local j-tiles for layer 1
BN_EPS = 1e-5

_CACHE = {}


def _build():
    nc = bacc.Bacc("TRN2", target_bir_lowering=False, debug=False,
                   num_devices=NCORES)

    xt_d = nc.dram_tensor("xt33", [33, N], BF16, kind="ExternalInput")
    xst_d = nc.dram_tensor("xst33", [33, RPC], BF16, kind="ExternalInput")
    w0all_d = nc.dram_tensor("w0all", [33, 80], BF16, kind="ExternalInput")
    w0s_d = nc.dram_tensor("w0s", [33, 8], BF16, kind="ExternalInput")
    w1allh_d = nc.dram_tensor("w1allh", [8, 8 * 34], BF16, kind="ExternalInput")
    w1ones_d = nc.dram_tensor("w1ones", [1, 34], BF16, kind="ExternalInput")
    sela_d = nc.dram_tensor("sela", [8, 8 * 128], BF16, kind="ExternalInput")
    out_d = nc.dram_tensor("out", [32, RPC], F32, kind="ExternalOutput")

    with tile.TileContext(nc) as tc:
        with (
            tc.tile_pool(name="const", bufs=1) as const,
            tc.tile_pool(name="persist", bufs=1) as per,
            tc.tile_pool(name="dram", bufs=1, space="DRAM") as dram,
        ):
            # warmup fodder memsets come absolutely first so the PE
            # warm-up burst can start immediately
            wsrc = const.tile([128, 512], BF16)
            nc.gpsimd.memset(wsrc[:], 0.5)
            wlhs = const.tile([128, 128], BF16)
            nc.gpsimd.memset(wlhs[:], 0.25)
            ones_row = const.tile([1, 128], F32)
            nc.vector.memset(ones_row[:], 1.0)
            ones_row_bf = const.tile([1, 128], BF16)
            nc.vector.memset(ones_row_bf[:], 1.0)
            neg1c = const.tile([8, 1], F32)
            nc.vector.memset(neg1c[:], -1.0)
            ident = const.tile([128, 128], F32)
            sela = const.tile([8, 8 * 128], BF16)
            nc.sync.dma_start(sela[:], sela_d[:])

            w0all = const.tile([33, 80], BF16)
            nc.sync.dma_start(w0all[:], w0all_d[:])
            w0s = const.tile([33, 8], BF16)
            nc.sync.dma_start(w0s[:], w0s_d[:])
            w1allh = const.tile([8, 8, 34], BF16)
            nc.sync.dma_start(
                w1allh[:], w1allh_d[:].rearrange("p (h c) -> p h c", h=8))
            w1ones = const.tile([1, 34], BF16)
            nc.sync.dma_start(w1ones[:], w1ones_d[:])

            # big persistent sbuf tensors
            xT = per.tile([33, N], BF16)       # x^T plus ones row
            xsT = per.tile([33, RPC], BF16)    # x_slice^T plus ones row
            # layer-0 stationary per (jt, h): scaled-hi 0:8, e^{d} at 32
            hpa0 = per.tile([128, NJT, 8, 33], BF16)
            d0r = per.tile([128, NJT, 8], F32)       # e^{-0.8 d0}
            atile = per.tile([128, 8, 512], BF16)    # e^{0.8 s0} bcast
            contc = per.tile([8, 8, 512], BF16)      # h1 local: [o, h, i]
            nrm = per.tile([8, 2, 512], F32)         # per-chunk normalized
            eneg = per.tile([8, 2, 512], F32)
            den2 = [per.tile([1, 2, 512], F32, name=f"den2_{c}")
                    for c in range(4)]
            lnden = [per.tile([1, 2, 512], F32, name=f"lnden_{c}")
                     for c in range(4)]
            rden2 = [per.tile([1, 2, 512], BF16, name=f"rden2_{c}")
                     for c in range(4)]
            # layer-1 stationary per jt: scaled-hi 0:32, e^{d1} at 32
            stat1 = per.tile([128, NJT1, 33], BF16)
            d1r = per.tile([128, NJT1], F32)         # e^{-0.8 d1}
            atile1 = per.tile([128, 8, 512], BF16)   # e^{0.8 s1} bcast
            s1loc = per.tile([128, NJT1], F32)
            s1row = per.tile([NJT1, 128], F32)
            s1g = per.tile([1, 8, 512], F32)
            a1rows = per.tile([1, 8, 512], BF16)
            rsb = per.tile([33, 512], F32)
            rden1 = per.tile([1, 512], BF16)
            rscr1 = per.tile([1, 512], F32)
            lnden1 = per.tile([1, 512], F32)
            norm1 = per.tile([32, 512], F32)

            s1d = dram.tile([NJT1, 128], F32, name="s1d", tag="s1d")
            s1gd = dram.tile([NCORES * NJT1, 128], F32, name="s1gd",
                             tag="s1gd")
            rsin = dram.tile([NCORES * 33, 512], F32, name="rsin", tag="rsin")
            rsout = dram.tile([33, 512], F32, name="rsout", tag="rsout")

            # ---------------- Phase A: projections -----------------
            with (
                tc.tile_pool(name="ld", bufs=2) as ld,
                tc.tile_pool(name="mm80", bufs=2, space="PSUM") as mm80,
                tc.tile_pool(name="pssa0", bufs=1, space="PSUM") as pssa0,
                tc.tile_pool(name="pssa", bufs=2, space="PSUM") as pssa,
            ):
                # PE warm-up burst: back-to-back matmuls flip the HAM
                # clock gate to 8/8 while input DMAs are still in flight
                wps = pssa0.tile([128, 512], F32, tag="wps")
                for r in range(20):
                    nc.tensor.matmul(wps[:], wlhs[:], wsrc[:],
                                     start=(r == 0), stop=(r == 19))

                nc.sync.dma_start(xT[:], xt_d[:])
                nc.sync.dma_start(xsT[:], xst_d[:])

                # s0 for this core's 512 dst rows; atile = e^{0.8 s0} bcast
                ps0 = pssa0.tile([8, 512], F32, tag="ps0")
                nc.tensor.matmul(ps0[:], w0s[:], xsT[:])
                a0row = ld.tile([8, 512], BF16, tag="a0row")
                nc.scalar.activation(a0row[:], ps0[:], ACT.Exp, scale=0.8)
                for h in range(8):
                    pa = pssa.tile([128, 512], F32, tag="pa")
                    nc.tensor.matmul(pa[:], sela[:, h * 128:(h + 1) * 128],
                                     a0row[:])
                    nc.scalar.copy(atile[:, h, :], pa[:])

                # h'0 scaled by e^{d0}, d0 exps, per 4-jt group
                for g in range(NJT // 4):
                    p80 = mm80.tile([128, 4, 80], F32)
                    for k in range(4):
                        jt = g * 4 + k
                        nc.tensor.matmul(p80[:, k, :],
                                         xT[:, jt * 128:(jt + 1) * 128],
                                         w0all[:])
                    # e^{d0} -> stationary col 32 (bf16), e^{-0.8 d0} -> d0r
                    nc.scalar.activation(
                        hpa0[:, g * 4:(g + 1) * 4, :, 32:33],
                        p80[:, :, 64:72], ACT.Exp)
                    nc.scalar.activation(
                        d0r[:, g * 4:(g + 1) * 4, :],
                        p80[:, :, 64:72], ACT.Exp, scale=-0.8)
                    for k in range(4):
                        jt = g * 4 + k
                        hsrc = p80[:, k, 0:64].rearrange("p (h o) -> p h o",
                                                         h=8)
                        sc_in, sc_b = broadcast_tensor_aps(
                            hsrc, hpa0[:, jt, :, 32:33])
                        nc.vector.tensor_tensor(hpa0[:, jt, :, 0:8],
                                                sc_in, sc_b, op=ALU.mult)

            # ------- Phase B: layer-0 attention + local normalize -------
            with (
                tc.tile_pool(name="epool", bufs=32) as epool,
                tc.tile_pool(name="nchunk", bufs=1) as nchunk,
                tc.tile_pool(name="agg", bufs=3, space="PSUM") as agg,
                tc.tile_pool(name="prb", bufs=1, space="PSUM") as prb,
                tc.tile_pool(name="p34p", bufs=1, space="PSUM") as p34p,
                tc.tile_pool(name="kap0", bufs=1, space="PSUM") as kap0,
            ):
                p34 = p34p.tile([128, NJT1, 34], F32, tag="p34")
                masks.make_identity(nc, ident[:])

                def keepalive(n, dep_ap):
                    ka = nchunk.tile([1, 128], BF16, tag="ka")
                    nc.scalar.copy(ka[:], dep_ap)
                    kps = kap0.tile([128, 512], F32, tag="kps")
                    for r in range(n):
                        nc.tensor.matmul(kps[:], ka[:], wsrc[0:1, :],
                                         start=(r == 0), stop=(r == n - 1))
                pgs = {}
                p34_started = [False]

                def norm_part_a(ch):
                    # only the reciprocal chain: runs on SE+DVE, nothing
                    # queued on the PE yet
                    pg_e, pg_o = pgs[2 * ch], pgs[2 * ch + 1]
                    nc.scalar.copy(den2[ch][:, 0, :], pg_e[32:33, :])
                    nc.scalar.copy(den2[ch][:, 1, :], pg_o[32:33, :])
                    nc.scalar.activation(lnden[ch][:], den2[ch][:], ACT.Ln)
                    nc.scalar.activation(rden2[ch][:], lnden[ch][:],
                                         ACT.Exp, scale=-1.0)

                def norm_part_b(ch):
                    # everything downstream of the reciprocal; issued one
                    # head later so the prb matmuls never block the PE
                    pg_e, pg_o = pgs[2 * ch], pgs[2 * ch + 1]
                    prb_e = prb.tile([8, 512], F32)
                    nc.tensor.matmul(prb_e[:], ones_row_bf[0:1, 0:8],
                                     rden2[ch][:, 0, :])
                    prb_o = prb.tile([8, 512], F32)
                    nc.tensor.matmul(prb_o[:], ones_row_bf[0:1, 0:8],
                                     rden2[ch][:, 1, :])
                    numc = nchunk.tile([8, 2, 512], F32, tag="numc")
                    nc.scalar.copy(numc[:, 0, :], pg_e[0:8, :])
                    nc.scalar.copy(numc[:, 1, :], pg_o[0:8, :])
                    nc.vector.tensor_tensor(nrm[:, 0, :], numc[:, 0, :],
                                            prb_e[:], op=ALU.mult)
                    nc.vector.tensor_tensor(nrm[:, 1, :], numc[:, 1, :],
                                            prb_o[:], op=ALU.mult)
                    # elu = (exp(-relu(-x)) - 1) + relu(x)
                    nc.scalar.activation(eneg[:], nrm[:], ACT.Relu,
                                         scale=-1.0)
                    nc.scalar.activation(eneg[:], eneg[:], ACT.Exp,
                                         scale=-1.0)
                    ppos = nchunk.tile([8, 2, 512], F32, tag="ppos")
                    nc.scalar.activation(ppos[:], nrm[:], ACT.Relu)
                    esum = nchunk.tile([8, 2, 512], F32, tag="esum")
                    nc.gpsimd.tensor_tensor(esum[:], eneg[:], ppos[:],
                                            op=ALU.add)
                    nc.scalar.activation(contc[:, 2 * ch:2 * ch + 2, :],
                                         esum[:], ACT.Identity,
                                         bias=neg1c[:])

                def p34_mms(ch):
                    # layer-1 projections (incl s1 col 33); single bank,
                    # single global start, region-wise stop; issued late
                    # enough that contc(ch) is long since ready
                    for jt in range(NJT1):
                        if ch == 0:
                            st = not p34_started[0]
                            p34_started[0] = True
                            nc.tensor.matmul(
                                p34[:, jt, :], ones_row_bf[:],
                                w1ones[:], start=st, stop=False)
                        for hc in (2 * ch, 2 * ch + 1):
                            nc.tensor.matmul(
                                p34[:, jt, :],
                                contc[:, hc, jt * 128:(jt + 1) * 128],
                                w1allh[:, hc, :],
                                start=False, stop=(ch == 3 and
                                                   hc == 2 * ch + 1))

                for h in range(8):
                    ch, hh = h // 2, h % 2
                    pg = agg.tile([33, 512], F32)
                    pgs[h] = pg
                    for jt in range(NJT):
                        if hh == 0 and h >= 2 and jt == 12:
                            norm_part_b(ch - 1)
                        if hh == 1 and h >= 3 and jt == 8:
                            p34_mms(ch - 1)
                        e = epool.tile([128, 512], BF16, tag="e")
                        nc.vector.tensor_scalar_max(
                            e[:], atile[:, h, :], d0r[:, jt, h:h + 1])
                        nc.tensor.matmul(pg[:], hpa0[:, jt, h, 0:33], e[:],
                                         start=(jt == 0), stop=(jt == NJT - 1))
                    if hh == 1:
                        norm_part_a(ch)
                        if ch == 3:
                            keepalive(14, den2[3][:, 0, 0:128])
                norm_part_b(3)
                p34_mms(3)

                # s1 row extraction: col 33 of p34 -> [4,128] row-major,
                # then a contiguous AllGather
                for jt in range(NJT1):
                    nc.scalar.copy(s1loc[:, jt:jt + 1], p34[:, jt, 33:34])
                ts1 = prb.tile([NJT1, 128], F32, tag="ts1")
                nc.tensor.matmul(ts1[:], s1loc[:], ident[:],
                                 is_transpose=True)
                nc.scalar.copy(s1row[:], ts1[:])
                nc.sync.dma_start(s1d[:], s1row[:])
                keepalive(40, s1row[0:1, :])
                nc.gpsimd.collective_compute(
                    "AllGather",
                    ALU.bypass,
                    replica_groups=[list(range(NCORES))],
                    ins=[s1d.opt()],
                    outs=[s1gd.opt()],
                )
                nc.sync.dma_start(s1g[:], s1gd[:].rearrange("a b -> (a b)"))

                # layer-1 stationary: scaled-hi + e^{d1} col, d1r
                for jt in range(NJT1):
                    nc.scalar.activation(stat1[:, jt, 32:33],
                                         p34[:, jt, 32:33], ACT.Exp)
                    nc.scalar.activation(d1r[:, jt:jt + 1],
                                         p34[:, jt, 32:33],
                                         ACT.Exp, scale=-0.8)
                    sc_in, sc_b = broadcast_tensor_aps(
                        p34[:, jt, 0:32], stat1[:, jt, 32:33])
                    nc.vector.tensor_tensor(stat1[:, jt, 0:32],
                                            sc_in, sc_b, op=ALU.mult)

            # ---------------- Phase D: layer 1 ----------------
            with (
                tc.tile_pool(name="e1pool", bufs=16) as e1pool,
                tc.tile_pool(name="pa1p", bufs=2, space="PSUM") as pa1p,
                tc.tile_pool(name="agg1", bufs=2, space="PSUM") as agg1,
                tc.tile_pool(name="prb1p", bufs=1, space="PSUM") as prb1p,
                tc.tile_pool(name="rslp", bufs=2) as rslp,
            ):
                nc.scalar.activation(a1rows[:], s1g[:], ACT.Exp, scale=0.8)
                for c in range(8):
                    pa1 = pa1p.tile([128, 512], F32)
                    nc.tensor.matmul(pa1[:], ones_row_bf[:],
                                     a1rows[:, c, :])
                    nc.scalar.copy(atile1[:, c, :], pa1[:])

                for c in range(8):
                    pg1 = agg1.tile([33, 512], F32)
                    for jt in range(NJT1):
                        e1 = e1pool.tile([128, 512], BF16, tag="e1")
                        nc.vector.tensor_scalar_max(
                            e1[:], atile1[:, c, :], d1r[:, jt:jt + 1])
                        nc.tensor.matmul(pg1[:], stat1[:, jt, 0:33], e1[:],
                                         start=(jt == 0),
                                         stop=(jt == NJT1 - 1))
                    rsl = rslp.tile([33, 512], F32, tag="rsl")
                    nc.scalar.copy(rsl[:], pg1[:])
                    nc.sync.dma_start(rsin[c * 33:(c + 1) * 33, :], rsl[:])

                nc.gpsimd.collective_compute(
                    "ReduceScatter",
                    ALU.add,
                    replica_groups=[list(range(NCORES))],
                    ins=[rsin.opt()],
                    outs=[rsout.opt()],
                )
                nc.sync.dma_start(rsb[:], rsout[:])
                nc.scalar.copy(rscr1[:], rsb[32:33, :])
                nc.scalar.activation(lnden1[:], rscr1[:], ACT.Ln)
                nc.scalar.activation(rden1[:], lnden1[:], ACT.Exp,
                                     scale=-1.0)
                prb1 = prb1p.tile([32, 512], F32, tag="prb1")
                nc.tensor.matmul(prb1[:], ones_row_bf[0:1, 0:32], rden1[:])
                nc.vector.tensor_tensor(norm1[:], rsb[0:32, :], prb1[:],
                                        op=ALU.mult)
                nc.sync.dma_start(out_d[:], norm1[:])

    nc.compile()
    return nc


def _fold(inputs):
    """Host-side BN folding and attention-projection folding (numpy)."""
    f64 = np.float64
    x = np.asarray(inputs["x"], np.float32)
    w0 = np.asarray(inputs["w0"], f64)          # [8, 32, 8]
    w1 = np.asarray(inputs["w1"], f64)          # [1, 64, 32]
    a_src0 = np.asarray(inputs["a_src0"], f64)[..., 0]   # [8, 8]
    a_dst0 = np.asarray(inputs["a_dst0"], f64)[..., 0]   # [8, 8]
    a_src1 = np.asarray(inputs["a_src1"], f64)[0, :, 0]  # [32]
    a_dst1 = np.asarray(inputs["a_dst1"], f64)[0, :, 0]  # [32]

    al0 = np.asarray(inputs["bn0_gamma"], f64) / np.sqrt(
        np.asarray(inputs["bn0_var"], f64) + BN_EPS)
    sh0 = np.asarray(inputs["bn0_beta"], f64) - \
        np.asarray(inputs["bn0_mean"], f64) * al0
    al1 = np.asarray(inputs["bn1_gamma"], f64) / np.sqrt(
        np.asarray(inputs["bn1_var"], f64) + BN_EPS)
    sh1 = np.asarray(inputs["bn1_beta"], f64) - \
        np.asarray(inputs["bn1_mean"], f64) * al1

    # layer 0 folds
    w0flat = (al0[None, :, None] * w0).transpose(1, 0, 2).reshape(32, 64)
    beta0h = np.einsum("i,hio->ho", sh0, w0)     # [8, 8]
    beta0 = beta0h.reshape(64)
    as0 = al0[:, None] * np.einsum("hio,ho->ih", w0, a_src0)   # [32, 8]
    sb0 = np.einsum("ho,ho->h", beta0h, a_src0)
    ad0 = al0[:, None] * np.einsum("hio,ho->ih", w0, a_dst0)
    db0 = np.einsum("ho,ho->h", beta0h, a_dst0)

    w0all = np.zeros((33, 80), f64)
    w0all[0:32, 0:64] = w0flat
    w0all[32, 0:64] = beta0
    w0all[0:32, 64:72] = ad0
    w0all[32, 64:72] = db0
    w0s = np.zeros((33, 8), f64)
    w0s[0:32, :] = as0
    w0s[32, :] = sb0

    # layer 1 folds (feature order f = h*8 + o to match contc [o, h, i])
    w1m = w1[0]                                   # [64, 32]
    w1flat = al1[:, None] * w1m
    beta1 = sh1 @ w1m                             # [32]
    as1 = al1 * (w1m @ a_src1)
    sb1 = beta1 @ a_src1
    ad1 = al1 * (w1m @ a_dst1)
    db1 = beta1 @ a_dst1

    w1allh = np.zeros((8, 8, 34), f64)            # [o, h, col]
    for h in range(8):
        w1allh[:, h, 0:32] = w1flat[h * 8:(h + 1) * 8]
        w1allh[:, h, 32] = ad1[h * 8:(h + 1) * 8]
        w1allh[:, h, 33] = as1[h * 8:(h + 1) * 8]
    w1ones = np.zeros((1, 34), f64)
    w1ones[0, 0:32] = beta1
    w1ones[0, 32] = db1
    w1ones[0, 33] = sb1

    sela = np.zeros((8, 8, 128), ml_dtypes.bfloat16)  # row h ones in block h
    for h in range(8):
        sela[h, h, :] = 1.0

    xt33 = np.concatenate(
        [x, np.ones((x.shape[0], 1), np.float32)], 1).T

    bf = ml_dtypes.bfloat16
    return {
        "xt33": np.ascontiguousarray(xt33.astype(bf)),
        "w0all": w0all.astype(bf),
        "w0s": w0s.astype(bf),
        "w1allh": w1allh.reshape(8, 8 * 34).astype(bf),
        "w1ones": w1ones.astype(bf),
        "sela": sela.reshape(8, 8 * 128),
    }


def kernel(**inputs) -> np.ndarray:
    if "nc" not in _CACHE:
        _CACHE["nc"] = _build()
    nc = _CACHE["nc"]

    shared = _fold(inputs)
    xt33 = shared["xt33"]
    in_maps = []
    for c in range(NCORES):
        m = dict(shared)
        m["xst33"] = np.ascontiguousarray(xt33[:, c * RPC:(c + 1) * RPC])
        in_maps.append(m)

    res = run_bass_kernel_spmd(nc, in_maps, list(range(NCORES)))
    out = np.concatenate(
        [np.ascontiguousarray(res.results[c]["out"].T)
         for c in range(NCORES)], axis=0)
    return out.astype(np.float32)
